# revision 18
# baseline (speedup 1.0000x reference)
"""HAWQ tiny classifier on 8 TRN2 cores — pure data parallel, v2.

Per core: batch shard [2048, 2000].  Key differences vs v1:
  - sig tiles are PE-transposed into a feature-major f32 buffer DURING the
    load phase (no DRAM bounce, no DMA-transpose).
  - quantize (magic-number round) runs per k-chunk after AllGather #1,
    alternating ACT/DVE, pipelined with the GEMM1 accumulation.
  - BN batch stats are sums of the UNROUNDED scaled activations (error
    ~1e-4), sent in the SAME AllGather as max(relu) -> 3 collectives total.
  - output is written [2, 2048] contiguous and transposed on the host
    (the strided [2048, 2] DMA write cost ~57us in v1).
Rounding uses the f32 magic-number trick (+1.5*2^23, RNE, matching
jnp.round for these ranges).
"""

import os
import sys

for p in ("/opt/trn_rl_repo", "/opt/trn_rl_repo/concourse"):
    if p not in sys.path:
        sys.path.insert(0, p)

import numpy as np
import ml_dtypes

import concourse.bass as bass
import concourse.bacc as bacc
import concourse.tile as tile
import concourse.mybir as mybir
from concourse import bass_utils
from concourse._compat import with_exitstack

F32 = mybir.dt.float32
BF16 = mybir.dt.bfloat16
AX = mybir.AxisListType.X
ALU = mybir.AluOpType
AF = mybir.ActivationFunctionType

BATCH, D_IN, HID, OUT = 16384, 2000, 100, 2
NCORES = 8
SHARD = BATCH // NCORES          # 2048 rows per core
NT = SHARD // 128                # 16 batch tiles per core
NK = (D_IN + 127) // 128         # 16 k-chunks (last one 80 rows)
LASTK = D_IN - 128 * (NK - 1)    # 80
MAGIC = 12582912.0               # 1.5 * 2**23
BN_EPS = 1e-5

_CACHE = {}


def _build(w1s: float, w2s: float):
    nc = bacc.Bacc(
        "TRN2",
        target_bir_lowering=False,
        debug=False,
        enable_asserts=False,
        num_devices=NCORES,
    )

    sig = nc.dram_tensor("sig", [SHARD, D_IN], F32, kind="ExternalInput")
    w1t = nc.dram_tensor("w1t", [D_IN, HID], BF16, kind="ExternalInput")
    w2t = nc.dram_tensor("w2t", [HID, OUT], BF16, kind="ExternalInput")
    b1i_t = nc.dram_tensor("b1i", [HID, 1], F32, kind="ExternalInput")
    qsc_t = nc.dram_tensor("qsc", [128, 1], F32, kind="ExternalInput")
    gmax_t = nc.dram_tensor("gmax", [1, 1], F32, kind="ExternalInput")
    b2 = nc.dram_tensor("b2", [OUT, 1], F32, kind="ExternalInput")
    gam = nc.dram_tensor("gamma", [HID, 1], F32, kind="ExternalInput")
    bet = nc.dram_tensor("beta", [HID, 1], F32, kind="ExternalInput")
    ident = nc.dram_tensor("ident", [128, 128], BF16, kind="ExternalInput")
    out = nc.dram_tensor("out", [OUT, SHARD], F32, kind="ExternalOutput")

    rg = [list(range(NCORES))]

    with tile.TileContext(nc) as tc:
        _kern(tc, nc, sig, w1t, w2t, b1i_t, qsc_t, gmax_t, b2, gam, bet,
              ident, out, rg, w1s, w2s)
    nc.compile()
    return nc


@with_exitstack
def _kern(ctx, tc, nc, sig, w1t, w2t, b1i_t, qsc_t, gmax_t, b2, gam, bet,
          ident, out, rg, w1s, w2s):
    sTp = ctx.enter_context(tc.tile_pool(name="sTp", bufs=1))   # big qT
    sigp = ctx.enter_context(tc.tile_pool(name="sigp", bufs=4))
    tmpp = ctx.enter_context(tc.tile_pool(name="tmpp", bufs=2))
    qbp = ctx.enter_context(tc.tile_pool(name="qbp", bufs=16))
    wp = ctx.enter_context(tc.tile_pool(name="wp", bufs=1))
    hp = ctx.enter_context(tc.tile_pool(name="hp", bufs=1))
    sp = ctx.enter_context(tc.tile_pool(name="sp", bufs=1))
    ptp = ctx.enter_context(tc.tile_pool(name="ptp", bufs=2, space="PSUM"))
    psb = ctx.enter_context(tc.tile_pool(name="psb", bufs=1, space="PSUM"))
    pss = ctx.enter_context(tc.tile_pool(name="pss", bufs=2, space="PSUM"))
    dcc = ctx.enter_context(tc.tile_pool(name="dcc", bufs=1, space="DRAM"))

    # ---- DMA issue: identity + quant scale + sig tiles, Sync queue ----
    id_sb = sp.tile([128, 128], BF16, tag="ident")
    nc.sync.dma_start(id_sb[:], ident[:, :])
    qsc = sp.tile([128, 1], F32, tag="qsc")
    nc.sync.dma_start(qsc[:], qsc_t[:, :])
    sigts = []
    for t in range(NT):
        st = sigp.tile([128, D_IN], F32, tag="sig")
        nc.sync.dma_start(st[:], sig[t * 128:(t + 1) * 128, :])
        sigts.append(st)

    # ---- warmups: CC-core (dummy collective), gpsimd MAX ucode, Sqrt table
    din0 = dcc.tile([1, 1], F32, tag="di0")
    dout0 = dcc.tile([NCORES, 1], F32, tag="do0")
    # no input DMA: contents are irrelevant, and a data dependency would
    # delay the trigger — this must fire immediately to warm the CC mesh
    nc.gpsimd.collective_compute(
        "AllGather", ALU.bypass, replica_groups=rg,
        ins=[din0.opt()], outs=[dout0.opt()])
    wrm = sp.tile([1, 1], F32, tag="wrm")
    nc.gpsimd.reduce_max(wrm[:], id_sb[0:2, 0:2],
                         axis=mybir.AxisListType.XYZWC)
    wrs = sp.tile([1, 1], F32, tag="wrs")
    nc.scalar.sqrt(wrs[:], id_sb[0:1, 0:1])

    # ---- constants on the Sync queue (after the sig loads) ----
    b1i = sp.tile([HID, 1], F32, tag="b1i")
    nc.sync.dma_start(b1i[:], b1i_t[:, :])
    gmax = sp.tile([1, 1], F32, tag="gmax")
    nc.sync.dma_start(gmax[:], gmax_t[:, :])
    b2_sb = sp.tile([OUT, 1], F32, tag="b2")
    nc.sync.dma_start(b2_sb[:], b2[:, :])
    gam_sb = sp.tile([HID, 1], F32, tag="gam")
    nc.sync.dma_start(gam_sb[:], gam[:, :])
    bet_sb = sp.tile([HID, 1], F32, tag="bet")
    nc.sync.dma_start(bet_sb[:], bet[:, :])
    w2t_sb = sp.tile([HID, OUT], BF16, tag="w2t")
    nc.sync.dma_start(w2t_sb[:], w2t[:, :])
    w1c = []
    for k in range(NK):
        parts = 128 if k < NK - 1 else LASTK
        wt = wp.tile([parts, HID], BF16, tag=f"w1_{k}")
        nc.gpsimd.dma_start(wt[:], w1t[k * 128:k * 128 + parts, :])
        w1c.append(wt)
    # constant rows for PE-matmul broadcasts (value * [1,1] scalar -> [n,1])
    rows = {}
    for val, tag in ((127.0, "r127"),
                     (w1s / (15.0 * 127.0), "rs2c"),
                     (w1s * w2s / (15.0 * 127.0), "rsw"),
                     (1.0 / w2s, "riw2"), (1.0 / 127.0, "ri127")):
        rt = sp.tile([1, 128], F32, tag=tag)
        nc.vector.memset(rt[:], float(val))
        rows[tag] = rt

    # ---- phase 1: quantize (host-provided scale) + bf16 PE transposes ----
    qT = sTp.tile([128, NK * SHARD], BF16, tag="qT")        # 64KB/partition
    qT_r = qT[:, :].rearrange("p (k b) -> p k b", k=NK)
    for t in range(NT):
        st = sigts[t]
        tmp = tmpp.tile([128, SHARD], F32, tag="tmp")
        nc.scalar.activation(tmp[:, 0:D_IN], st[:], AF.Copy, bias=MAGIC,
                             scale=qsc[:])
        qb = qbp.tile([128, D_IN], BF16, tag="qb")
        nc.vector.tensor_scalar_sub(qb[:], tmp[:, 0:D_IN], MAGIC)
        # chunks 0-7 -> one [128,1024] bf16 PSUM group, one copy
        pt = ptp.tile([128, 1024], BF16, tag="pt")
        for j in range(8):
            nc.tensor.transpose(pt[:, j * 128:(j + 1) * 128],
                                qb[:, j * 128:(j + 1) * 128], id_sb[:])
        nc.vector.tensor_copy(
            qT_r[:, 0:8, t * 128:(t + 1) * 128], pt[:])
        # chunks 8-10 (full) + chunk 15 (80 rows) on the PE
        pt = ptp.tile([128, 1024], BF16, tag="pt")
        for j in range(3):
            nc.tensor.transpose(pt[:, j * 128:(j + 1) * 128],
                                qb[:, (8 + j) * 128:(9 + j) * 128], id_sb[:])
        nc.tensor.transpose(pt[0:LASTK, 384:512], qb[:, 1920:D_IN],
                            id_sb[:])
        nc.vector.tensor_copy(
            qT_r[:, 8:11, t * 128:(t + 1) * 128], pt[:, 0:384])
        base = (NK - 1) * SHARD + t * 128
        nc.vector.tensor_copy(qT[0:LASTK, base:base + 128],
                              pt[0:LASTK, 384:512])
        # chunks 11-14 via the DMA crossbar transpose (idle DMA engines)
        for k in range(11, 15):
            nc.sync.dma_start(
                qT[:, k * SHARD + t * 128:k * SHARD + (t + 1) * 128],
                qb[:, k * 128:(k + 1) * 128], transpose=True)

    def bcast(scal, n, row, tag):
        """[n,1] = row_val * scal via a tiny PE matmul (no gpsimd ucode)."""
        ps = pss.tile([n, 1], F32, tag="ps")
        nc.tensor.matmul(ps[:], rows[row][0:1, 0:n], scal[:],
                         start=True, stop=True)
        r = sp.tile([n, 1], F32, tag=tag)
        nc.vector.tensor_copy(r[:], ps[:])
        return r

    # ---- phase 2: GEMM1 accumulation over the 16 k-chunks ----
    ps_a1 = psb.tile([HID, SHARD], F32, tag="big")
    for k in range(NK):
        parts = 128 if k < NK - 1 else LASTK
        for n in range(SHARD // 512):
            nc.tensor.matmul(
                ps_a1[:, n * 512:(n + 1) * 512], w1c[k][:],
                qT[0:parts, k * SHARD + n * 512:k * SHARD + (n + 1) * 512],
                start=(k == 0), stop=(k == NK - 1))

    # ---- relu (accum gives sum r) + local stats (max r, sum r^2) ----
    r = hp.tile([HID, SHARD], F32, tag="r")
    st3 = sp.tile([HID, 3], F32, tag="st3")
    nc.scalar.activation(r[:], ps_a1[:], AF.Relu, bias=b1i[:], scale=1.0,
                         accum_out=st3[:, 1:2])
    nc.vector.reduce_max(st3[:, 0:1], r[:], axis=AX)
    dum = tmpp.tile([128, SHARD], F32, tag="tmp")
    nc.scalar.activation(dum[0:HID, :], r[:], AF.Square,
                         accum_out=st3[:, 2:3])

    # ---- AllGather #2: [100,3] per core -> [800,3] ----
    din3 = dcc.tile([HID, 3], F32, tag="di3")
    dout3 = dcc.tile([HID * NCORES, 3], F32, tag="do3")
    nc.sync.dma_start(din3[:], st3[:])
    nc.gpsimd.collective_compute(
        "AllGather", ALU.bypass, replica_groups=rg,
        ins=[din3.opt()], outs=[dout3.opt()])
    # read back interleaved: gt[p, 3c+j] = dout3[100c+p, j]
    gt = sp.tile([HID, 3 * NCORES], F32, tag="gt")
    nc.sync.dma_start(
        gt[:, :].rearrange("p (c j) -> p c j", j=3),
        dout3[:, :].rearrange("(c p) j -> p c j", p=HID))
    rm_h = sp.tile([HID, 1], F32, tag="rm_h")
    nc.vector.reduce_max(rm_h[:], gt[:, 0:3 * NCORES:3], axis=AX)
    s1col = sp.tile([HID, 1], F32, tag="s1col")
    nc.vector.reduce_sum(s1col[:], gt[:, 1:3 * NCORES:3], axis=AX)
    s2col = sp.tile([HID, 1], F32, tag="s2col")
    nc.vector.reduce_sum(s2col[:], gt[:, 2:3 * NCORES:3], axis=AX)
    gmaxr = sp.tile([1, 1], F32, tag="gmaxr")
    nc.gpsimd.reduce_max(gmaxr[:], rm_h[:], axis=mybir.AxisListType.XYZWC)

    # ---- q2 quantize (both passes on ACT; DVE handles coefficients) ----
    rrm = sp.tile([1, 1], F32, tag="rrm")
    nc.vector.reciprocal(rrm[:], gmaxr[:])
    qsc2 = bcast(rrm, HID, "r127", "qsc2")         # [100,1] = 127/maxr = c
    nc.scalar.activation(r[:], r[:], AF.Copy, bias=MAGIC, scale=qsc2[:])

    # ---- BN coefficients from approximate integer-domain stats ----
    inv_n = 1.0 / float(BATCH)
    muq = sp.tile([HID, 1], F32, tag="muq")
    nc.vector.tensor_scalar(muq[:], s1col[:], qsc2[:], inv_n,
                            ALU.mult, ALU.mult)
    cb2 = sp.tile([HID, 1], F32, tag="cb2")
    nc.vector.tensor_tensor(cb2[:], qsc2[:], qsc2[:], ALU.mult)
    e2 = sp.tile([HID, 1], F32, tag="e2")
    nc.vector.tensor_scalar(e2[:], s2col[:], cb2[:], inv_n,
                            ALU.mult, ALU.mult)
    mq2 = sp.tile([HID, 1], F32, tag="mq2")
    nc.vector.tensor_tensor(mq2[:], muq[:], muq[:], ALU.mult)
    varq = sp.tile([HID, 1], F32, tag="varq")
    nc.vector.tensor_tensor(varq[:], e2[:], mq2[:], ALU.subtract)

    s2c = w1s / (15.0 * 127.0)
    pm = sp.tile([1, 1], F32, tag="pm")
    nc.vector.tensor_tensor(pm[:], gmaxr[:], gmax[:], ALU.mult)
    s2b = bcast(pm, HID, "rs2c", "s2b")              # [100,1] = s2
    s2sq = sp.tile([HID, 1], F32, tag="s2sq")
    nc.vector.tensor_tensor(s2sq[:], s2b[:], s2b[:], ALU.mult)
    var = sp.tile([HID, 1], F32, tag="var")
    nc.vector.tensor_scalar(var[:], varq[:], s2sq[:], BN_EPS,
                            ALU.mult, ALU.add)
    sd = sp.tile([HID, 1], F32, tag="sd")
    nc.scalar.sqrt(sd[:], var[:])
    q2 = hp.tile([HID, SHARD], BF16, tag="q2")
    nc.scalar.activation(q2[:], r[:], AF.Copy, bias=-MAGIC, scale=1.0)
    isd = sp.tile([HID, 1], F32, tag="isd")
    nc.vector.reciprocal(isd[:], sd[:])
    abn = sp.tile([HID, 1], F32, tag="abn")
    nc.vector.tensor_tensor(abn[:], gam_sb[:], isd[:], ALU.mult)
    mu = sp.tile([HID, 1], F32, tag="mu")
    nc.vector.tensor_tensor(mu[:], muq[:], s2b[:], ALU.mult)
    amu = sp.tile([HID, 1], F32, tag="amu")
    nc.vector.tensor_tensor(amu[:], abn[:], mu[:], ALU.mult)
    cbn = sp.tile([HID, 1], F32, tag="cbn")
    nc.vector.tensor_tensor(cbn[:], bet_sb[:], amu[:], ALU.subtract)
    abns = sp.tile([HID, 1], F32, tag="abns")
    nc.vector.tensor_scalar(abns[:], abn[:], s2b[:], w2s, ALU.mult, ALU.mult)

    def split2(src, n, tag):
        """src [n,1] f32 -> 2 (bf16, f32) pairs summing to ~src."""
        outs = []
        rem = src
        for j in range(2):
            h = sp.tile([n, 1], BF16, tag=f"{tag}_h{j}")
            nc.vector.tensor_copy(h[:], rem[:])
            f = sp.tile([n, 1], F32, tag=f"{tag}_f{j}")
            nc.vector.tensor_copy(f[:], h[:])
            outs.append((h, f))
            if j == 0:
                r2 = sp.tile([n, 1], F32, tag=f"{tag}_r{j}")
                nc.vector.tensor_tensor(r2[:], rem[:], f[:], ALU.subtract)
                rem = r2
        return outs

    ah = split2(abns, HID, "ah")
    weffs = []
    for j in range(2):
        wj = sp.tile([HID, OUT], BF16, tag=f"weff{j}")
        nc.vector.tensor_scalar_mul(wj[:], w2t_sb[:], ah[j][1][:])
        weffs.append(wj)
    ch = split2(cbn, HID, "ch")
    ps_zb = pss.tile([OUT, 1], F32, tag="ps")
    for j in range(2):
        nc.tensor.matmul(ps_zb[:], w2t_sb[:], ch[j][0][:],
                         start=(j == 0), stop=(j == 1))
    zb1 = sp.tile([OUT, 1], F32, tag="zb1")
    nc.vector.tensor_scalar_mul(zb1[:], ps_zb[:], w2s)
    s2_sc = sp.tile([1, 1], F32, tag="s2sc")
    nc.vector.tensor_scalar_mul(s2_sc[:], pm[:], s2c)      # [1,1] s2
    rs2 = sp.tile([1, 1], F32, tag="rs2")
    nc.vector.reciprocal(rs2[:], s2_sc[:])
    b2sc2 = bcast(rs2, OUT, "riw2", "b2sc2")            # [2,1] 1/(w2s*s2)
    t3 = sp.tile([OUT, 1], F32, tag="t3")
    nc.scalar.activation(t3[:], b2_sb[:], AF.Copy, bias=MAGIC, scale=b2sc2[:])
    b2i = sp.tile([OUT, 1], F32, tag="b2i")
    nc.vector.tensor_scalar(b2i[:], t3[:], MAGIC, 1.0, ALU.subtract, ALU.min)
    nc.vector.tensor_scalar_max(b2i[:], b2i[:], -2.0)
    sw2 = bcast(pm, OUT, "rsw", "sw2")                 # [2,1] s2*w2s
    b2is = sp.tile([OUT, 1], F32, tag="b2is")
    nc.vector.tensor_tensor(b2is[:], b2i[:], sw2[:], ALU.mult)
    zb2 = sp.tile([OUT, 1], F32, tag="zb2")
    nc.vector.tensor_tensor(zb2[:], zb1[:], b2is[:], ALU.add)

    # ---- GEMM2 (2 exact bf16 terms) + relu ----
    ps_z = psb.tile([OUT, SHARD], F32, tag="big")
    for n in range(SHARD // 512):
        for j in range(2):
            nc.tensor.matmul(ps_z[:, n * 512:(n + 1) * 512], weffs[j][:],
                             q2[:, n * 512:(n + 1) * 512],
                             start=(j == 0), stop=(j == 1))
    zr = hp.tile([OUT, SHARD], F32, tag="r")
    hh = SHARD // 2
    zm = sp.tile([OUT, 2], F32, tag="zm")
    nc.scalar.activation(zr[:, 0:hh], ps_z[:, 0:hh], AF.Relu, bias=zb2[:],
                         scale=1.0)
    nc.vector.reduce_max(zm[:, 0:1], zr[:, 0:hh], axis=AX)
    nc.scalar.activation(zr[:, hh:SHARD], ps_z[:, hh:SHARD], AF.Relu,
                         bias=zb2[:], scale=1.0)
    nc.vector.reduce_max(zm[:, 1:2], zr[:, hh:SHARD], axis=AX)

    # ---- AllGather #3: global max of relu(z) ----
    lmz = sp.tile([1, 1], F32, tag="lmz")
    nc.gpsimd.reduce_max(lmz[:], zm[:], axis=mybir.AxisListType.XYZWC)
    din4 = dcc.tile([1, 1], F32, tag="di4")
    dout4 = dcc.tile([NCORES, 1], F32, tag="do4")
    nc.sync.dma_start(din4[:], lmz[:])
    nc.gpsimd.collective_compute(
        "AllGather", ALU.bypass, replica_groups=rg,
        ins=[din4.opt()], outs=[dout4.opt()])
    g4 = sp.tile([NCORES, 1], F32, tag="g4")
    nc.sync.dma_start(g4[:], dout4[:])
    gmaxz = sp.tile([1, 1], F32, tag="gmaxz")
    nc.gpsimd.reduce_max(gmaxz[:], g4[:], axis=mybir.AxisListType.XYZWC)

    # ---- final 8-bit quant + dequant, output [2, 2048] ----
    rmz = sp.tile([1, 1], F32, tag="rmz")
    nc.vector.reciprocal(rmz[:], gmaxz[:])
    qsc3 = bcast(rmz, OUT, "r127", "qsc3")
    s3b = bcast(gmaxz, OUT, "ri127", "s3b")
    h = SHARD // 2
    nc.scalar.activation(zr[:, 0:h], zr[:, 0:h], AF.Copy, bias=MAGIC,
                         scale=qsc3[:])
    nc.vector.tensor_scalar(zr[:, h:SHARD], zr[:, h:SHARD], qsc3[:], MAGIC,
                            ALU.mult, ALU.add)
    nc.vector.tensor_scalar(zr[:, 0:h], zr[:, 0:h], MAGIC, s3b[:],
                            ALU.subtract, ALU.mult)
    nc.scalar.activation(zr[:, h:SHARD], zr[:, h:SHARD], AF.Copy,
                         bias=-MAGIC, scale=1.0)
    nc.vector.tensor_scalar_mul(zr[:, h:SHARD], zr[:, h:SHARD], s3b[:])
    nc.sync.dma_start(out[:, :], zr[:])


def _prep(sig, W1, b1, W2, b2, gamma, beta):
    sig = np.ascontiguousarray(np.asarray(sig, dtype=np.float32))
    W1 = np.asarray(W1, dtype=np.float32)
    W2 = np.asarray(W2, dtype=np.float32)
    w1s = float(np.max(np.abs(W1)))
    w2s = float(np.max(np.abs(W2)))
    w1i = np.clip(np.round(W1 / w1s), -2, 1).astype(np.float32)
    w2i = np.clip(np.round(W2 / w2s), -2, 1).astype(np.float32)
    w1t = np.ascontiguousarray(w1i.T).astype(ml_dtypes.bfloat16)
    w2t = np.ascontiguousarray(w2i.T).astype(ml_dtypes.bfloat16)
    gmax = float(np.max(np.abs(sig)))
    b1f = np.asarray(b1, np.float32).reshape(HID, 1)
    b1i = np.clip(np.round(b1f * (15.0 / (w1s * gmax))), -2.0, 1.0)
    com = {
        "w1t": w1t,
        "w2t": w2t,
        "b1i": b1i.astype(np.float32),
        "qsc": np.full((128, 1), 15.0 / gmax, dtype=np.float32),
        "gmax": np.full((1, 1), gmax, dtype=np.float32),
        "b2": np.ascontiguousarray(np.asarray(b2, np.float32).reshape(OUT, 1)),
        "gamma": np.asarray(gamma, np.float32).reshape(HID, 1),
        "beta": np.asarray(beta, np.float32).reshape(HID, 1),
        "ident": np.eye(128, dtype=ml_dtypes.bfloat16),
    }
    in_maps = []
    for c in range(NCORES):
        m = dict(com)
        m["sig"] = np.ascontiguousarray(sig[c * SHARD:(c + 1) * SHARD])
        in_maps.append(m)
    return w1s, w2s, in_maps


def kernel(sig, W1, b1, W2, b2, gamma, beta):
    w1s, w2s, in_maps = _prep(sig, W1, b1, W2, b2, gamma, beta)
    key = (round(w1s, 9), round(w2s, 9))
    if key not in _CACHE:
        _CACHE[key] = _build(w1s, w2s)
    nc = _CACHE[key]
    trace = os.environ.get("BASS_TRACE") == "1"
    try:
        res = bass_utils.run_bass_kernel_spmd(
            nc, in_maps, core_ids=list(range(NCORES)), trace=trace)
    except ModuleNotFoundError:
        os.environ["BASS_NEVER_TRACE"] = "1"
        res = bass_utils.run_bass_kernel_spmd(
            nc, in_maps, core_ids=list(range(NCORES)), trace=False)
    kernel.last_results = res
    return np.ascontiguousarray(
        np.concatenate([np.asarray(r["out"]).T for r in res.results], axis=0))


# revision 20
# speedup vs baseline: 1.6288x; 1.6288x over previous
"""HAWQ tiny classifier on 8 TRN2 cores — pure data parallel, v2.

Per core: batch shard [2048, 2000].  Key differences vs v1:
  - sig tiles are PE-transposed into a feature-major f32 buffer DURING the
    load phase (no DRAM bounce, no DMA-transpose).
  - quantize (magic-number round) runs per k-chunk after AllGather #1,
    alternating ACT/DVE, pipelined with the GEMM1 accumulation.
  - BN batch stats are sums of the UNROUNDED scaled activations (error
    ~1e-4), sent in the SAME AllGather as max(relu) -> 3 collectives total.
  - output is written [2, 2048] contiguous and transposed on the host
    (the strided [2048, 2] DMA write cost ~57us in v1).
Rounding uses the f32 magic-number trick (+1.5*2^23, RNE, matching
jnp.round for these ranges).
"""

import os
import sys

for p in ("/opt/trn_rl_repo", "/opt/trn_rl_repo/concourse"):
    if p not in sys.path:
        sys.path.insert(0, p)

import numpy as np
import ml_dtypes

import concourse.bass as bass
import concourse.bacc as bacc
import concourse.tile as tile
import concourse.mybir as mybir
from concourse import bass_utils
from concourse._compat import with_exitstack

F32 = mybir.dt.float32
BF16 = mybir.dt.bfloat16
AX = mybir.AxisListType.X
ALU = mybir.AluOpType
AF = mybir.ActivationFunctionType

BATCH, D_IN, HID, OUT = 16384, 2000, 100, 2
NCORES = 8
SHARD = BATCH // NCORES          # 2048 rows per core
NT = SHARD // 128                # 16 batch tiles per core
NK = (D_IN + 127) // 128         # 16 k-chunks (last one 80 rows)
LASTK = D_IN - 128 * (NK - 1)    # 80
MAGIC = 12582912.0               # 1.5 * 2**23
BN_EPS = 1e-5

_CACHE = {}


def _build(w1s: float, w2s: float):
    nc = bacc.Bacc(
        "TRN2",
        target_bir_lowering=False,
        debug=False,
        enable_asserts=False,
        num_devices=NCORES,
    )

    sig = nc.dram_tensor("sig", [SHARD, D_IN], F32, kind="ExternalInput")
    w1t = nc.dram_tensor("w1t", [D_IN, HID], BF16, kind="ExternalInput")
    w2t = nc.dram_tensor("w2t", [HID, OUT], BF16, kind="ExternalInput")
    b1i_t = nc.dram_tensor("b1i", [HID, 1], F32, kind="ExternalInput")
    qsc_t = nc.dram_tensor("qsc", [128, 1], F32, kind="ExternalInput")
    gmax_t = nc.dram_tensor("gmax", [1, 1], F32, kind="ExternalInput")
    b2 = nc.dram_tensor("b2", [OUT, 1], F32, kind="ExternalInput")
    gam = nc.dram_tensor("gamma", [HID, 1], F32, kind="ExternalInput")
    bet = nc.dram_tensor("beta", [HID, 1], F32, kind="ExternalInput")
    ident = nc.dram_tensor("ident", [128, 128], BF16, kind="ExternalInput")
    out = nc.dram_tensor("out", [OUT, SHARD], F32, kind="ExternalOutput")

    rg = [list(range(NCORES))]

    with tile.TileContext(nc) as tc:
        _kern(tc, nc, sig, w1t, w2t, b1i_t, qsc_t, gmax_t, b2, gam, bet,
              ident, out, rg, w1s, w2s)
    nc.compile()
    return nc


@with_exitstack
def _kern(ctx, tc, nc, sig, w1t, w2t, b1i_t, qsc_t, gmax_t, b2, gam, bet,
          ident, out, rg, w1s, w2s):
    sTp = ctx.enter_context(tc.tile_pool(name="sTp", bufs=1))   # big qT
    sigp = ctx.enter_context(tc.tile_pool(name="sigp", bufs=4))
    tmpp = ctx.enter_context(tc.tile_pool(name="tmpp", bufs=3))
    qbp = ctx.enter_context(tc.tile_pool(name="qbp", bufs=16))
    wp = ctx.enter_context(tc.tile_pool(name="wp", bufs=1))
    hp = ctx.enter_context(tc.tile_pool(name="hp", bufs=1))
    sp = ctx.enter_context(tc.tile_pool(name="sp", bufs=1))
    ptp = ctx.enter_context(tc.tile_pool(name="ptp", bufs=2, space="PSUM"))
    psb = ctx.enter_context(tc.tile_pool(name="psb", bufs=1, space="PSUM"))
    pss = ctx.enter_context(tc.tile_pool(name="pss", bufs=2, space="PSUM"))
    dcc = ctx.enter_context(tc.tile_pool(name="dcc", bufs=1, space="DRAM"))

    # ---- DMA issue: identity + quant scale + sig tiles, Sync queue ----
    id_sb = sp.tile([128, 128], BF16, tag="ident")
    nc.sync.dma_start(id_sb[:], ident[:, :])
    qsc = sp.tile([128, 1], F32, tag="qsc")
    nc.sync.dma_start(qsc[:], qsc_t[:, :])
    sigts = []
    for t in range(NT):
        st = sigp.tile([128, D_IN], F32, tag="sig")
        nc.sync.dma_start(st[:], sig[t * 128:(t + 1) * 128, :])
        sigts.append(st)

    # ---- warmups: CC-core (dummy collective), gpsimd MAX ucode, Sqrt table
    din0 = dcc.tile([1, 1], F32, tag="di0")
    dout0 = dcc.tile([NCORES, 1], F32, tag="do0")
    # no input DMA: contents are irrelevant, and a data dependency would
    # delay the trigger — this must fire immediately to warm the CC mesh
    nc.gpsimd.collective_compute(
        "AllGather", ALU.bypass, replica_groups=rg,
        ins=[din0.opt()], outs=[dout0.opt()])
    wrm = sp.tile([1, 1], F32, tag="wrm")
    nc.gpsimd.reduce_max(wrm[:], id_sb[0:2, 0:2],
                         axis=mybir.AxisListType.XYZWC)
    wrs = sp.tile([1, 1], F32, tag="wrs")
    nc.scalar.sqrt(wrs[:], id_sb[0:1, 0:1])

    # ---- constants on the Sync queue (after the sig loads) ----
    b1i = sp.tile([HID, 1], F32, tag="b1i")
    nc.sync.dma_start(b1i[:], b1i_t[:, :])
    gmax = sp.tile([1, 1], F32, tag="gmax")
    nc.sync.dma_start(gmax[:], gmax_t[:, :])
    b2_sb = sp.tile([OUT, 1], F32, tag="b2")
    nc.sync.dma_start(b2_sb[:], b2[:, :])
    gam_sb = sp.tile([HID, 1], F32, tag="gam")
    nc.sync.dma_start(gam_sb[:], gam[:, :])
    bet_sb = sp.tile([HID, 1], F32, tag="bet")
    nc.sync.dma_start(bet_sb[:], bet[:, :])
    w2t_sb = sp.tile([HID, OUT], BF16, tag="w2t")
    nc.sync.dma_start(w2t_sb[:], w2t[:, :])
    w1c = []
    for k in range(NK):
        parts = 128 if k < NK - 1 else LASTK
        wt = wp.tile([parts, HID], BF16, tag=f"w1_{k}")
        nc.sync.dma_start(wt[:], w1t[k * 128:k * 128 + parts, :])
        w1c.append(wt)
    # constant rows for PE-matmul broadcasts (value * [1,1] scalar -> [n,1])
    rows = {}
    for val, tag in ((127.0, "r127"),
                     (w1s / (15.0 * 127.0), "rs2c"),
                     (w1s * w2s / (15.0 * 127.0), "rsw"),
                     (1.0 / w2s, "riw2"), (1.0 / 127.0, "ri127")):
        rt = sp.tile([1, 128], F32, tag=tag)
        nc.vector.memset(rt[:], float(val))
        rows[tag] = rt

    # ---- phase 1: quantize (host-provided scale) + bf16 PE transposes ----
    qT = sTp.tile([128, NK * SHARD], BF16, tag="qT")        # 64KB/partition
    qT_r = qT[:, :].rearrange("p (k b) -> p k b", k=NK)
    for t in range(NT):
        st = sigts[t]
        tmp = tmpp.tile([128, SHARD], F32, tag="tmp")
        nc.scalar.activation(tmp[:, 0:D_IN], st[:], AF.Copy, bias=MAGIC,
                             scale=qsc[:])
        qb = qbp.tile([128, D_IN], BF16, tag="qb")
        nc.vector.tensor_scalar_sub(qb[:], tmp[:, 0:D_IN], MAGIC)
        # chunks 0-7 -> one [128,1024] bf16 PSUM group, one copy
        pt = ptp.tile([128, 1024], BF16, tag="pt")
        for j in range(8):
            nc.tensor.transpose(pt[:, j * 128:(j + 1) * 128],
                                qb[:, j * 128:(j + 1) * 128], id_sb[:])
        nc.vector.tensor_copy(
            qT_r[:, 0:8, t * 128:(t + 1) * 128], pt[:])
        # chunks 8-14 (full) + chunk 15 (80 rows)
        pt = ptp.tile([128, 1024], BF16, tag="pt")
        for j in range(7):
            nc.tensor.transpose(pt[:, j * 128:(j + 1) * 128],
                                qb[:, (8 + j) * 128:(9 + j) * 128], id_sb[:])
        nc.tensor.transpose(pt[0:LASTK, 896:1024], qb[:, 1920:D_IN],
                            id_sb[:])
        nc.vector.tensor_copy(
            qT_r[:, 8:15, t * 128:(t + 1) * 128], pt[:, 0:896])
        base = (NK - 1) * SHARD + t * 128
        nc.vector.tensor_copy(qT[0:LASTK, base:base + 128],
                              pt[0:LASTK, 896:1024])

    def bcast(scal, n, row, tag):
        """[n,1] = row_val * scal via a tiny PE matmul (no gpsimd ucode)."""
        ps = pss.tile([n, 1], F32, tag="ps")
        nc.tensor.matmul(ps[:], rows[row][0:1, 0:n], scal[:],
                         start=True, stop=True)
        r = sp.tile([n, 1], F32, tag=tag)
        nc.vector.tensor_copy(r[:], ps[:])
        return r

    # ---- phase 2: GEMM1 accumulation over the 16 k-chunks ----
    ps_a1 = psb.tile([HID, SHARD], F32, tag="big")
    for k in range(NK):
        parts = 128 if k < NK - 1 else LASTK
        for n in range(SHARD // 512):
            nc.tensor.matmul(
                ps_a1[:, n * 512:(n + 1) * 512], w1c[k][:],
                qT[0:parts, k * SHARD + n * 512:k * SHARD + (n + 1) * 512],
                start=(k == 0), stop=(k == NK - 1))

    # ---- relu (accum gives sum r) + local stats (max r, sum r^2) ----
    r = hp.tile([HID, SHARD], F32, tag="r")
    st3 = sp.tile([HID, 3], F32, tag="st3")
    nc.scalar.activation(r[:], ps_a1[:], AF.Relu, bias=b1i[:], scale=1.0,
                         accum_out=st3[:, 1:2])
    nc.vector.reduce_max(st3[:, 0:1], r[:], axis=AX)
    dum = tmpp.tile([128, SHARD], F32, tag="tmp")
    nc.scalar.activation(dum[0:HID, :], r[:], AF.Square,
                         accum_out=st3[:, 2:3])

    # ---- AllGather #2: [100,3] per core -> [800,3] ----
    din3 = dcc.tile([HID, 3], F32, tag="di3")
    dout3 = dcc.tile([HID * NCORES, 3], F32, tag="do3")
    nc.sync.dma_start(din3[:], st3[:])
    nc.gpsimd.collective_compute(
        "AllGather", ALU.bypass, replica_groups=rg,
        ins=[din3.opt()], outs=[dout3.opt()])
    # read back interleaved: gt[p, 3c+j] = dout3[100c+p, j]
    gt = sp.tile([HID, 3 * NCORES], F32, tag="gt")
    nc.sync.dma_start(
        gt[:, :].rearrange("p (c j) -> p c j", j=3),
        dout3[:, :].rearrange("(c p) j -> p c j", p=HID))
    rm_h = sp.tile([HID, 1], F32, tag="rm_h")
    nc.vector.reduce_max(rm_h[:], gt[:, 0:3 * NCORES:3], axis=AX)
    s1col = sp.tile([HID, 1], F32, tag="s1col")
    nc.vector.reduce_sum(s1col[:], gt[:, 1:3 * NCORES:3], axis=AX)
    s2col = sp.tile([HID, 1], F32, tag="s2col")
    nc.vector.reduce_sum(s2col[:], gt[:, 2:3 * NCORES:3], axis=AX)
    gmaxr = sp.tile([1, 1], F32, tag="gmaxr")
    nc.gpsimd.reduce_max(gmaxr[:], rm_h[:], axis=mybir.AxisListType.XYZWC)

    # ---- q2 quantize (both passes on ACT; DVE handles coefficients) ----
    rrm = sp.tile([1, 1], F32, tag="rrm")
    nc.vector.reciprocal(rrm[:], gmaxr[:])
    qsc2 = bcast(rrm, HID, "r127", "qsc2")         # [100,1] = 127/maxr = c
    nc.scalar.activation(r[:], r[:], AF.Copy, bias=MAGIC, scale=qsc2[:])

    # ---- BN coefficients from approximate integer-domain stats ----
    inv_n = 1.0 / float(BATCH)
    muq = sp.tile([HID, 1], F32, tag="muq")
    nc.vector.tensor_scalar(muq[:], s1col[:], qsc2[:], inv_n,
                            ALU.mult, ALU.mult)
    cb2 = sp.tile([HID, 1], F32, tag="cb2")
    nc.vector.tensor_tensor(cb2[:], qsc2[:], qsc2[:], ALU.mult)
    e2 = sp.tile([HID, 1], F32, tag="e2")
    nc.vector.tensor_scalar(e2[:], s2col[:], cb2[:], inv_n,
                            ALU.mult, ALU.mult)
    mq2 = sp.tile([HID, 1], F32, tag="mq2")
    nc.vector.tensor_tensor(mq2[:], muq[:], muq[:], ALU.mult)
    varq = sp.tile([HID, 1], F32, tag="varq")
    nc.vector.tensor_tensor(varq[:], e2[:], mq2[:], ALU.subtract)

    s2c = w1s / (15.0 * 127.0)
    pm = sp.tile([1, 1], F32, tag="pm")
    nc.vector.tensor_tensor(pm[:], gmaxr[:], gmax[:], ALU.mult)
    s2b = bcast(pm, HID, "rs2c", "s2b")              # [100,1] = s2
    s2sq = sp.tile([HID, 1], F32, tag="s2sq")
    nc.vector.tensor_tensor(s2sq[:], s2b[:], s2b[:], ALU.mult)
    var = sp.tile([HID, 1], F32, tag="var")
    nc.vector.tensor_scalar(var[:], varq[:], s2sq[:], BN_EPS,
                            ALU.mult, ALU.add)
    sd = sp.tile([HID, 1], F32, tag="sd")
    nc.scalar.sqrt(sd[:], var[:])
    q2 = hp.tile([HID, SHARD], BF16, tag="q2")
    nc.scalar.activation(q2[:], r[:], AF.Copy, bias=-MAGIC, scale=1.0)
    isd = sp.tile([HID, 1], F32, tag="isd")
    nc.vector.reciprocal(isd[:], sd[:])
    abn = sp.tile([HID, 1], F32, tag="abn")
    nc.vector.tensor_tensor(abn[:], gam_sb[:], isd[:], ALU.mult)
    mu = sp.tile([HID, 1], F32, tag="mu")
    nc.vector.tensor_tensor(mu[:], muq[:], s2b[:], ALU.mult)
    amu = sp.tile([HID, 1], F32, tag="amu")
    nc.vector.tensor_tensor(amu[:], abn[:], mu[:], ALU.mult)
    cbn = sp.tile([HID, 1], F32, tag="cbn")
    nc.vector.tensor_tensor(cbn[:], bet_sb[:], amu[:], ALU.subtract)
    abns = sp.tile([HID, 1], F32, tag="abns")
    nc.vector.tensor_scalar(abns[:], abn[:], s2b[:], w2s, ALU.mult, ALU.mult)

    def split2(src, n, tag):
        """src [n,1] f32 -> 2 (bf16, f32) pairs summing to ~src."""
        outs = []
        rem = src
        for j in range(2):
            h = sp.tile([n, 1], BF16, tag=f"{tag}_h{j}")
            nc.vector.tensor_copy(h[:], rem[:])
            f = sp.tile([n, 1], F32, tag=f"{tag}_f{j}")
            nc.vector.tensor_copy(f[:], h[:])
            outs.append((h, f))
            if j == 0:
                r2 = sp.tile([n, 1], F32, tag=f"{tag}_r{j}")
                nc.vector.tensor_tensor(r2[:], rem[:], f[:], ALU.subtract)
                rem = r2
        return outs

    ah = split2(abns, HID, "ah")
    weffs = []
    for j in range(2):
        wj = sp.tile([HID, OUT], BF16, tag=f"weff{j}")
        nc.vector.tensor_scalar_mul(wj[:], w2t_sb[:], ah[j][1][:])
        weffs.append(wj)
    ch = split2(cbn, HID, "ch")
    ps_zb = pss.tile([OUT, 1], F32, tag="ps")
    for j in range(2):
        nc.tensor.matmul(ps_zb[:], w2t_sb[:], ch[j][0][:],
                         start=(j == 0), stop=(j == 1))
    zb1 = sp.tile([OUT, 1], F32, tag="zb1")
    nc.vector.tensor_scalar_mul(zb1[:], ps_zb[:], w2s)
    s2_sc = sp.tile([1, 1], F32, tag="s2sc")
    nc.vector.tensor_scalar_mul(s2_sc[:], pm[:], s2c)      # [1,1] s2
    rs2 = sp.tile([1, 1], F32, tag="rs2")
    nc.vector.reciprocal(rs2[:], s2_sc[:])
    b2sc2 = bcast(rs2, OUT, "riw2", "b2sc2")            # [2,1] 1/(w2s*s2)
    t3 = sp.tile([OUT, 1], F32, tag="t3")
    nc.scalar.activation(t3[:], b2_sb[:], AF.Copy, bias=MAGIC, scale=b2sc2[:])
    b2i = sp.tile([OUT, 1], F32, tag="b2i")
    nc.vector.tensor_scalar(b2i[:], t3[:], MAGIC, 1.0, ALU.subtract, ALU.min)
    nc.vector.tensor_scalar_max(b2i[:], b2i[:], -2.0)
    sw2 = bcast(pm, OUT, "rsw", "sw2")                 # [2,1] s2*w2s
    b2is = sp.tile([OUT, 1], F32, tag="b2is")
    nc.vector.tensor_tensor(b2is[:], b2i[:], sw2[:], ALU.mult)
    zb2 = sp.tile([OUT, 1], F32, tag="zb2")
    nc.vector.tensor_tensor(zb2[:], zb1[:], b2is[:], ALU.add)

    # ---- GEMM2 (2 exact bf16 terms) + relu ----
    ps_z = psb.tile([OUT, SHARD], F32, tag="big")
    for n in range(SHARD // 512):
        for j in range(2):
            nc.tensor.matmul(ps_z[:, n * 512:(n + 1) * 512], weffs[j][:],
                             q2[:, n * 512:(n + 1) * 512],
                             start=(j == 0), stop=(j == 1))
    zr = hp.tile([OUT, SHARD], F32, tag="r")
    hh = SHARD // 2
    zm = sp.tile([OUT, 2], F32, tag="zm")
    nc.scalar.activation(zr[:, 0:hh], ps_z[:, 0:hh], AF.Relu, bias=zb2[:],
                         scale=1.0)
    nc.vector.reduce_max(zm[:, 0:1], zr[:, 0:hh], axis=AX)
    nc.scalar.activation(zr[:, hh:SHARD], ps_z[:, hh:SHARD], AF.Relu,
                         bias=zb2[:], scale=1.0)
    nc.vector.reduce_max(zm[:, 1:2], zr[:, hh:SHARD], axis=AX)

    # ---- AllGather #3: global max of relu(z) ----
    lmz = sp.tile([1, 1], F32, tag="lmz")
    nc.gpsimd.reduce_max(lmz[:], zm[:], axis=mybir.AxisListType.XYZWC)
    din4 = dcc.tile([1, 1], F32, tag="di4")
    dout4 = dcc.tile([NCORES, 1], F32, tag="do4")
    nc.sync.dma_start(din4[:], lmz[:])
    nc.gpsimd.collective_compute(
        "AllGather", ALU.bypass, replica_groups=rg,
        ins=[din4.opt()], outs=[dout4.opt()])
    g4 = sp.tile([NCORES, 1], F32, tag="g4")
    nc.sync.dma_start(g4[:], dout4[:])
    gmaxz = sp.tile([1, 1], F32, tag="gmaxz")
    nc.gpsimd.reduce_max(gmaxz[:], g4[:], axis=mybir.AxisListType.XYZWC)

    # ---- final 8-bit quant + dequant, output [2, 2048] ----
    rmz = sp.tile([1, 1], F32, tag="rmz")
    nc.vector.reciprocal(rmz[:], gmaxz[:])
    qsc3 = bcast(rmz, OUT, "r127", "qsc3")
    s3b = bcast(gmaxz, OUT, "ri127", "s3b")
    h = SHARD // 2
    nc.scalar.activation(zr[:, 0:h], zr[:, 0:h], AF.Copy, bias=MAGIC,
                         scale=qsc3[:])
    nc.vector.tensor_scalar(zr[:, h:SHARD], zr[:, h:SHARD], qsc3[:], MAGIC,
                            ALU.mult, ALU.add)
    nc.vector.tensor_scalar(zr[:, 0:h], zr[:, 0:h], MAGIC, s3b[:],
                            ALU.subtract, ALU.mult)
    nc.scalar.activation(zr[:, h:SHARD], zr[:, h:SHARD], AF.Copy,
                         bias=-MAGIC, scale=1.0)
    nc.vector.tensor_scalar_mul(zr[:, h:SHARD], zr[:, h:SHARD], s3b[:])
    nc.sync.dma_start(out[:, :], zr[:])


def _prep(sig, W1, b1, W2, b2, gamma, beta):
    sig = np.ascontiguousarray(np.asarray(sig, dtype=np.float32))
    W1 = np.asarray(W1, dtype=np.float32)
    W2 = np.asarray(W2, dtype=np.float32)
    w1s = float(np.max(np.abs(W1)))
    w2s = float(np.max(np.abs(W2)))
    w1i = np.clip(np.round(W1 / w1s), -2, 1).astype(np.float32)
    w2i = np.clip(np.round(W2 / w2s), -2, 1).astype(np.float32)
    w1t = np.ascontiguousarray(w1i.T).astype(ml_dtypes.bfloat16)
    w2t = np.ascontiguousarray(w2i.T).astype(ml_dtypes.bfloat16)
    gmax = float(np.max(np.abs(sig)))
    b1f = np.asarray(b1, np.float32).reshape(HID, 1)
    b1i = np.clip(np.round(b1f * (15.0 / (w1s * gmax))), -2.0, 1.0)
    com = {
        "w1t": w1t,
        "w2t": w2t,
        "b1i": b1i.astype(np.float32),
        "qsc": np.full((128, 1), 15.0 / gmax, dtype=np.float32),
        "gmax": np.full((1, 1), gmax, dtype=np.float32),
        "b2": np.ascontiguousarray(np.asarray(b2, np.float32).reshape(OUT, 1)),
        "gamma": np.asarray(gamma, np.float32).reshape(HID, 1),
        "beta": np.asarray(beta, np.float32).reshape(HID, 1),
        "ident": np.eye(128, dtype=ml_dtypes.bfloat16),
    }
    in_maps = []
    for c in range(NCORES):
        m = dict(com)
        m["sig"] = np.ascontiguousarray(sig[c * SHARD:(c + 1) * SHARD])
        in_maps.append(m)
    return w1s, w2s, in_maps


def kernel(sig, W1, b1, W2, b2, gamma, beta):
    w1s, w2s, in_maps = _prep(sig, W1, b1, W2, b2, gamma, beta)
    key = (round(w1s, 9), round(w2s, 9))
    if key not in _CACHE:
        _CACHE[key] = _build(w1s, w2s)
    nc = _CACHE[key]
    trace = os.environ.get("BASS_TRACE") == "1"
    try:
        res = bass_utils.run_bass_kernel_spmd(
            nc, in_maps, core_ids=list(range(NCORES)), trace=trace)
    except ModuleNotFoundError:
        os.environ["BASS_NEVER_TRACE"] = "1"
        res = bass_utils.run_bass_kernel_spmd(
            nc, in_maps, core_ids=list(range(NCORES)), trace=False)
    kernel.last_results = res
    return np.ascontiguousarray(
        np.concatenate([np.asarray(r["out"]).T for r in res.results], axis=0))
